# revision 18
# baseline (speedup 1.0000x reference)
"""Multi-head attention (B=8, S=2048, D=512, H=8, DH=64) on 8 TRN2 NeuronCores.

Strategy: data-parallel over the batch dim — core b computes batch element b
end-to-end (no collectives). Per core, everything is kept transposed
("feature on partitions") so that softmax reductions land on the TensorE
contraction axis:

  1. QKV projection with head-interleaved, pre-transposed weights gives
     Q^T, K^T laid out (64h+c, s) and V laid out (s, 64h+c). All inputs
     arrive as ONE wide-tile DMA each (3D access patterns) spread over
     four engine queues, so the first scores run ~16us in.
  2. Scores are computed transposed, S^T[j, i] = sum_c K^T[c,j] Q^T[c,i],
     as K=64 matmuls row-packed two-at-a-time into disjoint PE row groups
     (partitions 0:64 / 64:128 via lo/hi replicas of Q^T/K^T).
  3. Each head runs as TWO i-sweeps of 1024 queries. This halves the PV
     PSUM accumulator to 2 banks, which frees enough PSUM to give the
     score staging pool THREE 2-bank buffers; PV trails the score stream
     by TWO j-chunks so the exp latency (~1.3us) never stalls the PE.
  4. exp(scale * S^T) is split across TWO engines: ScalarE runs true exp
     out of PSUM; VectorE handles ~30% of chunks with a Schraudolph bit
     trick (y = s*A + B in f32, round-to-nearest to int16, bit-cast to
     bf16 == 2^(s*scale*log2e), ~3% piecewise-linear error that largely
     cancels under softmax normalization).
  5. O^T[c, i] = sum_j Vaug[j, c] E^T[j, i] with Vaug = [V | ones]: M=65
     matmuls whose 65th row accumulates the softmax denominator for free.
     V lives in ONE wide SBUF tile; each V-projection unit retires with a
     single 4D-strided cast so downstream PSUM-buffer reuse waits on one
     instruction.
  6. The V projection and the remaining QK projection chunks are
     interleaved into the head loops as PE filler; only the sweep-0
     halves of QK chunk 0 run before head 0.
  7. Normalization per (head, sweep) in bf16: denominator row into
     (128,8) for a cheap reciprocal + DRAM partition-broadcast; the
     normalizing multiply runs at DVE 2x (bf16) on VectorE for odd heads
     and on GpSimd for even heads. The last sweep's DMA chain uses the
     idle GpSimd queue.
"""

import numpy as np
import ml_dtypes

B, S, D = 8, 2048, 512
H, DH = 8, 64
INNER = H * DH
SCALE = DH ** -0.5

N_CORES = 8
NDT = D // 128   # 4 contraction tiles
NSC = S // 128   # 16 s-chunks (j-chunks)
SW = 1024        # i-sweep width
VW = DH + 1      # v_aug width per head
CW = H * VW      # v_aug width per j-chunk (520)

A_TRICK = float(SCALE * np.log2(np.e) * 128.0)
B_TRICK = float(127 * 128 - 8.0)
DVE_FRAC = 0.32
TRAIL = 2


def _build_kernel():
    import concourse.bass as bass
    import concourse.mybir as mybir
    import concourse.tile as tile
    from concourse import bacc

    bf16 = mybir.dt.bfloat16
    f32 = mybir.dt.float32
    i16 = mybir.dt.int16
    Exp = mybir.ActivationFunctionType.Exp

    nc = bacc.Bacc()

    xT = nc.declare_dram_parameter("xT", [D, S], bf16, isOutput=False)
    wq = nc.declare_dram_parameter("wq", [D, INNER], bf16, isOutput=False)
    wk = nc.declare_dram_parameter("wk", [D, INNER], bf16, isOutput=False)
    wv = nc.declare_dram_parameter("wv", [D, INNER], bf16, isOutput=False)
    wo = nc.declare_dram_parameter("wo", [INNER, D], bf16, isOutput=False)
    bo = nc.declare_dram_parameter("bo", [NDT, 128, 1], f32, isOutput=False)
    out = nc.declare_dram_parameter("out", [D, S], f32, isOutput=True)
    den_dram = nc.dram_tensor("den_scratch", [2 * H, SW], bf16)

    # ---- exp-engine assignment (one 128x1024 chunk per (h, sweep, jc)) ----
    forced_act = set()
    for jc in range(13, NSC):                # keep VectorE free for the tail
        forced_act.add((7, 1, jc))
    dve_steps = set()
    n_dve = 0
    n_seen = 0
    for hh in range(H):
        for sw in range(2):
            for jc in range(NSC):
                n_seen += 1
                if (hh, sw, jc) in forced_act:
                    continue
                if n_dve < DVE_FRAC * n_seen:
                    dve_steps.add((hh, sw, jc))
                    n_dve += 1

    with tile.TileContext(nc) as tc:
        with (
            tc.tile_pool(name="weights", bufs=1) as wpool,
            tc.tile_pool(name="acts", bufs=1) as apool,
            tc.tile_pool(name="et", bufs=5) as epool,
            tc.tile_pool(name="small", bufs=2) as spool,
            tc.tile_pool(name="ostage", bufs=2) as opool,
            tc.tile_pool(name="psA", bufs=3, space="PSUM") as psA,
            tc.tile_pool(name="psV", bufs=1, space="PSUM") as psV,
        ):
            junk_sb = wpool.tile([128, 512], bf16, name="junk", tag="junk")
            # ---- wide input tiles, spread over three DMA queues ----
            # xT_b columns: hf*4096 + d*1024 + c   (c in [0,1024))
            xT_b = wpool.tile([128, 2 * NDT * 1024], bf16, name="xTb",
                              tag="xTb")
            # w*_b columns: d*512 + e
            wq_b = wpool.tile([128, NDT * INNER], bf16, name="wqb", tag="wqb")
            wk_b = wpool.tile([128, NDT * INNER], bf16, name="wkb", tag="wkb")
            wv_b = wpool.tile([128, NDT * INNER], bf16, name="wvb", tag="wvb")
            wo_b = wpool.tile([128, NDT * D], bf16, name="wob", tag="wob")
            bo_b = wpool.tile([128, NDT], f32, name="bob", tag="bob")

            def wide_src(dram, row_pitch, inner):
                return bass.AP(
                    tensor=dram.tensor,
                    offset=dram.offset,
                    ap=[[row_pitch, 128], [128 * row_pitch, NDT],
                        [1, inner]],
                )

            def xv(d, hf):
                c0 = d * 2048 + hf * 1024
                return xT_b[:, c0:c0 + 1024]

            # junk-buffer memset first so PE warm-up isn't gated on DMAs
            nc.gpsimd.memset(junk_sb[:, :], 0.0)
            # xT: one full-row DMA per d-tile (4KB runs), two per queue
            for d, q in ((0, nc.sync), (1, nc.sync), (2, nc.scalar),
                         (3, nc.scalar)):
                q.dma_start(out=xT_b[:, d * 2048:(d + 1) * 2048],
                            in_=xT[d * 128:(d + 1) * 128, :])
            nc.sync.dma_start(out=wq_b[:].rearrange(
                "p (d e) -> p d e", d=NDT), in_=wide_src(wq[:, :], INNER,
                                                         INNER))
            nc.scalar.dma_start(out=wk_b[:].rearrange(
                "p (d e) -> p d e", d=NDT), in_=wide_src(wk[:, :], INNER,
                                                         INNER))
            nc.gpsimd.dma_start(out=wv_b[:].rearrange(
                "p (d e) -> p d e", d=NDT), in_=wide_src(wv[:, :], INNER,
                                                         INNER))
            nc.gpsimd.dma_start(out=wo_b[:].rearrange(
                "p (d e) -> p d e", d=NDT), in_=wide_src(wo[:, :], D, D))
            bo_ap = bo[0, :, :]
            nc.gpsimd.dma_start(
                out=bo_b[:, :],
                in_=bass.AP(tensor=bo_ap.tensor, offset=bo_ap.offset,
                            ap=[[1, 128], [128, NDT]]),
            )

            qt_lo = [apool.tile([128, S], bf16, name=f"qlo{t}", tag=f"qlo{t}")
                     for t in range(NDT)]
            kt_lo = [apool.tile([128, S], bf16, name=f"klo{t}", tag=f"klo{t}")
                     for t in range(NDT)]
            qt_hi = [apool.tile([128, S], bf16, name=f"qhi{t}", tag=f"qhi{t}")
                     for t in range(NDT)]
            kt_hi = [apool.tile([128, S], bf16, name=f"khi{t}", tag=f"khi{t}")
                     for t in range(NDT)]
            va_big = apool.tile([128, NSC * CW], bf16, name="vab", tag="vab")
            ot = [apool.tile([128, S], bf16, name=f"ot{t}", tag=f"ot{t}")
                  for t in range(NDT)]

            # PE warm-up during the input-DMA window (HAM un-throttle).
            junk_ps = psV.tile([128, 1024], f32, name="junkps", tag="pv")
            for k in range(44):
                nc.tensor.matmul(
                    junk_ps[:, (k % 2) * 512:(k % 2 + 1) * 512],
                    lhsT=junk_sb[:, 0:128],
                    rhs=junk_sb[:, :],
                )

            def qk_half(w_b, dst, ch, half):
                """One 1024-col half of a QK projection chunk: 8 matmuls
                (4 accum x 2 positions) + single PSUM->SBUF cast."""
                pa = psA.tile([128, 1024], f32, name="pa", tag="pa")
                for d in range(NDT):
                    for nn in range(2):
                        s0 = nn * 512
                        nc.tensor.matmul(
                            pa[:, nn * 512:(nn + 1) * 512],
                            lhsT=w_b[:, d * INNER + ch * 128:
                                     d * INNER + (ch + 1) * 128],
                            rhs=xv(d, half)[:, s0:s0 + 512],
                            start=(d == 0),
                            stop=(d == NDT - 1),
                        )
                nc.scalar.copy(
                    dst[ch][:, half * 1024:(half + 1) * 1024], pa[:, :])

            def swap_part(lo, hi, t, half):
                sl = slice(half * 1024, (half + 1) * 1024)
                nc.sync.dma_start(out=hi[t][64:128, sl], in_=lo[t][0:64, sl])
                nc.sync.dma_start(out=hi[t][0:64, sl], in_=lo[t][64:128, sl])

            def v_unit(u):
                """V projection for m-chunks 2u, 2u+1 via the psA pool;
                retires with ONE 4D cast + ONE strided ones-memset."""
                pvt = psA.tile([128, 1024], f32, name="pa", tag="pa")
                for k in range(2):
                    m = 2 * u + k
                    for d in range(NDT):
                        mh, mo = divmod(m, 8)
                        nc.tensor.matmul(
                            pvt[:, k * 512:(k + 1) * 512],
                            lhsT=xv(d, mh)[:, mo * 128:(mo + 1) * 128],
                            rhs=wv_b[:, d * INNER:(d + 1) * INNER],
                            start=(d == 0),
                            stop=(d == NDT - 1),
                        )
                dst = va_big[:, u * 2 * CW:(u + 1) * 2 * CW].rearrange(
                    "p (k h t) -> p k h t", k=2, t=VW)
                src = pvt[:, :].rearrange("p (k h t) -> p k h t", k=2, t=DH)
                nc.scalar.copy(dst[:, :, :, 0:DH], src)
                nc.vector.memset(dst[:, :, :, DH:VW], 1.0)

            # minimal pre-head-0 work
            qk_half(wq_b, qt_lo, 0, 0)
            qk_half(wk_b, kt_lo, 0, 0)
            swap_part(qt_lo, qt_hi, 0, 0)
            swap_part(kt_lo, kt_hi, 0, 0)

            # filler schedule: (h, sweep, jc) -> list of callables
            fillers = {}

            def k0_rest():
                qk_half(wk_b, kt_lo, 0, 1)
                swap_part(kt_lo, kt_hi, 0, 1)

            def q0_rest():
                qk_half(wq_b, qt_lo, 0, 1)
                swap_part(qt_lo, qt_hi, 0, 1)

            fillers[(0, 0, 0)] = [k0_rest]
            vslots = [1, 2, 4, 6, 8, 10, 12, 13]
            for u in range(8):
                fillers.setdefault((0, 0, vslots[u]), []).append(
                    lambda u=u: v_unit(u))
            fillers[(0, 0, 14)] = [q0_rest]
            for t2 in range(1, NDT):
                h_q, h_k = 2 * (t2 - 1), 2 * (t2 - 1) + 1
                fillers.setdefault((h_q, 1, 5), []).append(
                    lambda c=t2: qk_half(wq_b, qt_lo, c, 0))
                fillers.setdefault((h_q, 1, 11), []).append(
                    lambda c=t2: qk_half(wq_b, qt_lo, c, 1))
                fillers.setdefault((h_k, 0, 5), []).append(
                    lambda c=t2: qk_half(wk_b, kt_lo, c, 0))

                def k1_and_swap(c=t2):
                    qk_half(wk_b, kt_lo, c, 1)
                    for half in range(2):
                        swap_part(qt_lo, qt_hi, c, half)
                        swap_part(kt_lo, kt_hi, c, half)
                fillers.setdefault((h_k, 0, 11), []).append(k1_and_swap)

            # ---- attention: 2 sweeps of 1024 queries per head ----
            def norm_block(pv, h, sweep, t, p, i0, last):
                dma_q = nc.gpsimd if last else nc.sync
                oun = spool.tile([VW, SW], bf16, name="oun", tag="oun")
                if last:
                    nc.scalar.copy(oun[:, :], pv[0:VW, :])
                else:
                    nc.vector.tensor_copy(oun[:, :], pv[0:VW, :])
                den128 = spool.tile([128, 8], bf16, name="den128",
                                    tag="d128")
                dma_q.dma_start(out=den128[:, :], in_=oun[DH:DH + 1, :])
                with nc.allow_low_precision(
                        reason="bf16 softmax denominator; validated "
                               "within the 2e-2 tolerance"):
                    nc.vector.reciprocal(out=den128[:, :],
                                         in_=den128[:, :])
                row = 2 * h + sweep
                dma_q.dma_start(out=den_dram[row, :], in_=den128[:, :])
                bc = spool.tile([64, SW], bf16, name="bc", tag="bc")
                dd = den_dram[row:row + 1, :]
                bcast_src = bass.AP(
                    tensor=dd.tensor,
                    offset=dd.offset,
                    ap=[[0, 64]] + [list(x) for x in dd.ap[1:]],
                )
                dma_q.dma_start(out=bc[:, :], in_=bcast_src)
                mul_eng = nc.gpsimd if p == 0 else nc.vector
                mul_eng.tensor_mul(
                    ot[t][64 * p:64 * p + 64, i0:i0 + SW],
                    oun[0:DH, :], bc[:, :])

            pending_finish = [None]
            for h in range(H):
                t, p = h // 2, h % 2
                lo_sl = slice(64 * p, 64 * p + 64)
                hi_sl = slice(64 * (1 - p), 64 * (1 - p) + 64)
                for sweep in range(2):
                    i0 = sweep * SW
                    pv = psV.tile([128, 1024], f32, name="pvh", tag="pv")
                    ets = {}

                    def pv_mms(jc, pv=pv, ets=ets, h=h):
                        for it in range(2):
                            nc.tensor.matmul(
                                pv[0:VW, it * 512:(it + 1) * 512],
                                lhsT=va_big[:, jc * CW + h * VW:
                                            jc * CW + (h + 1) * VW],
                                rhs=ets[jc][:, it * 512:(it + 1) * 512],
                                start=(jc == 0),
                                stop=(jc == NSC - 1),
                            )

                    for jc in range(NSC):
                        et = epool.tile([128, 1024], bf16, name="et",
                                        tag="et")
                        ets[jc] = et
                        pa = psA.tile([128, 1024], f32, name="pa", tag="pa")
                        nc.tensor.matmul(
                            pa[:, 0:512],
                            lhsT=kt_lo[t][lo_sl, jc * 128:(jc + 1) * 128],
                            rhs=qt_lo[t][lo_sl, i0:i0 + 512],
                        )
                        nc.tensor.matmul(
                            pa[:, 512:1024],
                            lhsT=kt_hi[t][hi_sl, jc * 128:(jc + 1) * 128],
                            rhs=qt_hi[t][hi_sl, i0 + 512:i0 + 1024],
                        )
                        if (h, sweep, jc) in dve_steps:
                            nc.vector.tensor_scalar(
                                out=et[:, :].bitcast(i16),
                                in0=pa[:, :],
                                scalar1=A_TRICK,
                                scalar2=B_TRICK,
                                op0=mybir.AluOpType.mult,
                                op1=mybir.AluOpType.add,
                            )
                        else:
                            nc.scalar.activation(
                                out=et[:, :],
                                in_=pa[:, :],
                                func=Exp,
                                scale=SCALE,
                            )
                        if jc == 0 and pending_finish[0] is not None:
                            pending_finish[0]()
                            pending_finish[0] = None
                        if jc % 2 == 1 and jc >= TRAIL + 1:
                            pv_mms(jc - TRAIL - 1)
                            pv_mms(jc - TRAIL)
                        for f in fillers.get((h, sweep, jc), ()):
                            f()
                    last = (h == H - 1 and sweep == 1)

                    def finish(pv_mms=pv_mms, pv=pv, h=h, sweep=sweep,
                               t=t, p=p, i0=i0, last=last):
                        for jc2 in range(NSC - TRAIL, NSC):
                            pv_mms(jc2)
                        norm_block(pv, h, sweep, t, p, i0, last)

                    if last:
                        finish()
                    else:
                        pending_finish[0] = finish

            # ---- output projection (the scheduler hoists each group's
            # matmuls as soon as its ot slices are normalized; kt=3 last) ----
            for ch in range(NDT):
                for half in range(2):
                    stage = opool.tile([128, 1024], f32, name="stage",
                                       tag="stage")
                    po = psA.tile([128, 1024], f32, name="pa", tag="pa")
                    for st2 in range(2):
                        st = half * 2 + st2
                        for kt in range(NDT):
                            nc.tensor.matmul(
                                po[:, st2 * 512:(st2 + 1) * 512],
                                lhsT=wo_b[:, kt * D + ch * 128:
                                          kt * D + (ch + 1) * 128],
                                rhs=ot[kt][:, st * 512:(st + 1) * 512],
                                start=(kt == 0),
                                stop=(kt == NDT - 1),
                            )
                    if half == 1:
                        nc.vector.tensor_scalar_add(
                            out=stage[:, :], in0=po[:, :],
                            scalar1=bo_b[:, ch:ch + 1])
                    else:
                        nc.scalar.add(stage[:, :], po[:, :],
                                      bo_b[:, ch:ch + 1])
                    nc.sync.dma_start(
                        out=out[ch * 128:(ch + 1) * 128,
                                half * 1024:(half + 1) * 1024],
                        in_=stage[:, :],
                    )

    nc.finalize()
    return nc


_NC_CACHE = None


def _get_nc():
    global _NC_CACHE
    if _NC_CACHE is None:
        _NC_CACHE = _build_kernel()
    return _NC_CACHE


def kernel(x, W_qkv, W_out, b_out):
    from concourse.bass_utils import run_bass_kernel_spmd

    bf16 = ml_dtypes.bfloat16

    # head-interleave and transpose the qkv weight: row 192h+{0,64,128}+c of
    # W_qkv is q/k/v row (h, c); regroup to e' = 64h+c and transpose to [d, e']
    w3 = W_qkv.reshape(H, 3, DH, D)
    wq_h = np.ascontiguousarray(w3[:, 0].reshape(INNER, D).T).astype(bf16)
    wk_h = np.ascontiguousarray(w3[:, 1].reshape(INNER, D).T).astype(bf16)
    wv_h = np.ascontiguousarray(w3[:, 2].reshape(INNER, D).T).astype(bf16)
    wo_h = np.ascontiguousarray(W_out.T).astype(bf16)  # [hc, d]
    bo_h = np.ascontiguousarray(b_out.reshape(NDT, 128, 1)).astype(np.float32)

    in_maps = []
    for b in range(N_CORES):
        xT_b = np.ascontiguousarray(x[b].T).astype(bf16)  # [d, s]
        in_maps.append({
            "xT": xT_b, "wq": wq_h, "wk": wk_h, "wv": wv_h,
            "wo": wo_h, "bo": bo_h,
        })

    nc = _get_nc()
    res = run_bass_kernel_spmd(nc, in_maps, list(range(N_CORES)))
    outs = [res.results[b]["out"].T for b in range(N_CORES)]  # [s, d] each
    return np.ascontiguousarray(np.stack(outs, axis=0)).astype(np.float32)
